# revision 1
# baseline (speedup 1.0000x reference)
"""Trainium2 Bass kernel for nn_MemoryRetriever (cross-attention memory retriever).

Sharding: memory tokens (Sk=31290) split across 8 NeuronCores (3968/core,
zero-padded, padded keys masked off).  Each core computes K/V projections +
RMSNorm + 3D-RoPE for its key shard, full Q (redundant, tiny), local
masked-softmax partials (un-normalized numerator + denominator; no max
subtraction needed since |score| <~ 8), then one on-device AllReduce combines
partials and each core output-projects its own 64-query slice.  Host
concatenates the 8 slices.

All on-chip layouts are feature-major ([d, token]): projections, RoPE (pair
swap via a +-1 permutation matmul), RMSNorm (sum-of-squares via ones-vector
matmul), scores and attention*V run on the PE array with no transposes.
Matmul operands are bf16 (fp32 PSUM accumulation); softmax/normalization
arithmetic is fp32.
"""

import sys
import numpy as np

sys.path.insert(0, "/opt/trn_rl_repo")

DIM = 1024
HEADS = 8
HD = 128
SQ = 512
SK = 31290
N_CORES = 8
SKC = 3968           # keys per core (31 tiles of 128); 8*3968 = 31744 >= 31290
TT = SKC // 128
QS = SQ // N_CORES
EPS = 1e-6
SCALE = 1.0 / np.sqrt(128.0)
NEG = -1.0e30
CHUNK_TILES = 4      # key tiles per chunk

_cache = {}


def _build():
    if "nc" in _cache:
        return _cache["nc"]

    import concourse.bass as bass
    import concourse.tile as tile
    from concourse import mybir, bacc

    f32 = mybir.dt.float32
    bf16 = mybir.dt.bfloat16
    AF = mybir.ActivationFunctionType

    nc = bacc.Bacc("TRN2", target_bir_lowering=False, debug=False,
                   num_devices=N_CORES)  # _sim handled below

    def din(name, shape, dt=f32):
        return nc.dram_tensor(name, list(shape), dt, kind="ExternalInput").ap()

    # per-core sharded inputs
    memT = din("memT", [DIM, SKC], bf16)    # mem shard, feature-major, bf16
    ctk = din("ctk", [HD, SKC])             # K rope cos table (in-head d major)
    stk = din("stk", [HD, SKC])
    mbias = din("mbias", [128, TT])         # mask bias (0 / -1e30)
    # shared inputs
    xT = din("xT", [DIM, SQ], bf16)
    wq = din("wq", [128, 8, 8, 128], bf16)  # [p,i,o,m] = Wq.T[i*128+p, o*128+m]
    wk = din("wk", [128, 8, 8, 128], bf16)
    wo = din("wo", [128, 8, 8, 128], bf16)  # [p,o,e,m] = Wo.T[o*128+p, e*128+m]
    wv = din("wv", [128, 8, DIM], bf16)     # [p,i,o] = Wv.T[i*128+p, o]
    ctq = din("ctq", [128, 8, SQ])          # q rope cos (gq folded)
    stq = din("stq", [128, 8, SQ])
    bq_t = din("bq_t", [128, 8])
    bk_t = din("bk_t", [128, 8])
    bo_t = din("bo_t", [128, 8])
    bv_t = din("bv_t", [128, DIM])
    pmat = din("pmat", [128, 128], bf16)    # P.T for rope pair swap (+-1)
    ones_c = din("ones_c", [128, 1], bf16)
    ones_f = din("ones_f", [128, 1])
    eps_in = din("eps_c", [1, 1])

    outT = nc.dram_tensor("outT", [DIM, SQ], f32, kind="ExternalOutput").ap()

    import os as _os
    _dbg = _os.environ.get("KDBG", "0") == "1"
    _sim = _os.environ.get("KSIM", "0") == "1"
    if _dbg:
        qdbg = nc.dram_tensor("qdbg", [128, 8, SQ], mybir.dt.bfloat16, kind="ExternalOutput").ap()
        ykdbg = nc.dram_tensor("ykdbg", [128, 8, 256], f32, kind="ExternalOutput").ap()
        rsbdbg = nc.dram_tensor("rsbdbg", [128, 256], f32, kind="ExternalOutput").ap()
        krdbg = nc.dram_tensor("krdbg", [128, 8, 256], mybir.dt.bfloat16, kind="ExternalOutput").ap()
        dadbg = nc.dram_tensor("dadbg", [128, 8, SQ], f32, kind="ExternalOutput").ap()
        nadbg = nc.dram_tensor("nadbg", [128, 8, SQ], f32, kind="ExternalOutput").ap()
        dendbg = nc.dram_tensor("dendbg", [1, 8, SQ], f32, kind="ExternalOutput").ap()
        catdbg = nc.dram_tensor("catdbg", [DIM + HEADS, SQ], f32, kind="ExternalOutput").ap()
        catshdbg = nc.dram_tensor("catshdbg", [DIM + HEADS, SQ], f32, kind="ExternalOutput").ap()
    cat = nc.dram_tensor("cat", [DIM + HEADS, SQ], f32)
    cat_sh = nc.dram_tensor("cat_sh", [DIM + HEADS, SQ], f32, addr_space="Shared")

    with tile.TileContext(nc) as tc:
        ctx_pools = []

        def pool(name, bufs, space=None):
            kw = dict(name=name, bufs=bufs)
            if space:
                kw["space"] = space
            p = tc.tile_pool(**kw)
            ctx_pools.append(p)
            return p.__enter__()

        consts = pool("consts", 1)
        resid = pool("resid", 1)
        pp = pool("pp", 3, space="PSUM")
        pp_att = pool("pp_att", 4, space="PSUM")
        pp_sq = pool("pp_sq", 1, space="PSUM")

        # ---- constants / resident tensors ----
        pt_s = consts.tile([128, 128], bf16)
        nc.sync.dma_start(pt_s[:], pmat)
        ones_s = consts.tile([128, 1], bf16)
        nc.sync.dma_start(ones_s[:], ones_c)
        ones_fs = consts.tile([128, 1], f32)
        nc.sync.dma_start(ones_fs[:], ones_f)
        mb_s = consts.tile([128, TT], f32)
        nc.sync.dma_start(mb_s[:], mbias)
        bq_s = consts.tile([128, 8], f32)
        nc.sync.dma_start(bq_s[:], bq_t)
        bk_s = consts.tile([128, 8], f32)
        nc.sync.dma_start(bk_s[:], bk_t)
        bo_s = consts.tile([128, 8], f32)
        nc.sync.dma_start(bo_s[:], bo_t)
        bv_s = consts.tile([128, DIM], f32)
        nc.sync.dma_start(bv_s[:], bv_t)
        eps_s = consts.tile([1, 1], f32)
        nc.sync.dma_start(eps_s[:], eps_in)
        wk_s = resid.tile([128, 8, 8, 128], bf16)
        nc.sync.dma_start(wk_s[:], wk)
        wv_s = resid.tile([128, 8, DIM], bf16)
        nc.sync.dma_start(wv_s[:], wv)

        qT = resid.tile([128, 8, SQ], bf16)     # rope'd Q, feature-major
        nacc = resid.tile([128, 8, SQ], f32)    # numerator accumulator
        dacc = resid.tile([128, 8, SQ], f32)    # exp-sum accumulator

        # =========== Q phase ===========
        qpool_cm = tc.tile_pool(name="qpool", bufs=1)
        qpool = qpool_cm.__enter__()
        qpf_cm = tc.tile_pool(name="qpf", bufs=2)
        qpf = qpf_cm.__enter__()
        xt_s = qpool.tile([128, 8, SQ], bf16, tag="xt")
        nc.sync.dma_start(xt_s[:], xT.rearrange("(i p) q -> p i q", p=128))
        yq = qpool.tile([128, 8, SQ], f32, tag="yq")
        ybq = qpool.tile([128, 8, SQ], bf16, tag="ybq")
        ps_sq_q = pp_sq.tile([1, SQ], f32, tag="pssq")
        for o in range(8):
            wq_o = qpf.tile([128, 8, 128], bf16, tag="wq_o")
            nc.sync.dma_start(wq_o[:], wq[:, :, o, :])
            ps_q = pp.tile([128, SQ], f32, tag="ps")
            for i in range(8):
                nc.tensor.matmul(ps_q[:], wq_o[:, i, :], xt_s[:, i, :],
                                 start=(i == 0), stop=(i == 7))
            nc.scalar.activation(yq[:, o, :], ps_q[:], AF.Identity,
                                 bias=bq_s[:, o:o + 1])
            ysq = qpool.tile([128, SQ], bf16, tag="ysq")
            nc.vector.tensor_mul(ysq[:], yq[:, o, :], yq[:, o, :])
            nc.vector.tensor_copy(ybq[:, o, :], yq[:, o, :])
            nc.tensor.matmul(ps_sq_q[:], ones_s[:], ysq[:],
                             start=(o == 0), stop=(o == 7))
        sq_q = qpool.tile([1, SQ], f32, tag="sqr")
        nc.scalar.activation(sq_q[:], ps_sq_q[:], AF.Sqrt,
                             bias=eps_s[:], scale=1.0 / DIM)
        rs_q = qpool.tile([1, SQ], f32, tag="rs")
        nc.vector.reciprocal(rs_q[:], sq_q[:])
        rsb_q = qpool.tile([128, SQ], f32, tag="rsb")
        nc.gpsimd.partition_broadcast(rsb_q[:], rs_q[:])
        for o in range(8):
            ctq_o = qpf.tile([128, SQ], f32, tag="ctq_o")
            nc.sync.dma_start(ctq_o[:], ctq[:, o, :])
            stq_o = qpf.tile([128, SQ], f32, tag="stq_o")
            nc.sync.dma_start(stq_o[:], stq[:, o, :])
            ps_sw = pp.tile([128, SQ], f32, tag="ps")
            nc.tensor.matmul(ps_sw[:], pt_s[:], ybq[:, o, :])
            t1 = qpool.tile([128, SQ], f32, tag="t1")
            nc.vector.tensor_mul(t1[:], yq[:, o, :], ctq_o[:])
            t2 = qpool.tile([128, SQ], f32, tag="t2")
            nc.vector.tensor_mul(t2[:], ps_sw[:], stq_o[:])
            nc.vector.tensor_add(t1[:], t1[:], t2[:])
            nc.vector.tensor_mul(qT[:, o, :], t1[:], rsb_q[:])
        if _dbg:
            nc.sync.dma_start(qdbg, qT[:])
        qpf_cm.__exit__(None, None, None)
        qpool_cm.__exit__(None, None, None)

        # =========== main loop over key chunks ===========
        kpool_cm = tc.tile_pool(name="kpool", bufs=2)
        kpool = kpool_cm.__enter__()
        ppool_cm = tc.tile_pool(name="ppool", bufs=12)
        ppool = ppool_cm.__enter__()
        for ci, ct0 in enumerate(range(0, TT, CHUNK_TILES)):
            ntt = min(CHUNK_TILES, TT - ct0)
            cw = ntt * 128
            c0 = ct0 * 128
            memt = kpool.tile([128, 8, cw], bf16, tag="memt")
            nc.sync.dma_start(
                memt[:], memT[:, c0:c0 + cw].rearrange("(i p) t -> p i t", p=128))
            ctk_t = kpool.tile([128, cw], f32, tag="ctk")
            nc.sync.dma_start(ctk_t[:], ctk[:, c0:c0 + cw])
            stk_t = kpool.tile([128, cw], f32, tag="stk")
            nc.sync.dma_start(stk_t[:], stk[:, c0:c0 + cw])

            yk = kpool.tile([128, 8, cw], bf16, tag="yk")
            ps_sq = pp_sq.tile([1, cw], f32, tag="pssq")
            for o in range(8):
                ps_y = pp.tile([128, cw], f32, tag="ps")
                for i in range(8):
                    nc.tensor.matmul(ps_y[:], wk_s[:, i, o, :], memt[:, i, :],
                                     start=(i == 0), stop=(i == 7))
                nc.scalar.activation(yk[:, o, :], ps_y[:], AF.Identity,
                                     bias=bk_s[:, o:o + 1])
                ysq = kpool.tile([128, cw], bf16, tag="ysq")
                nc.vector.tensor_mul(ysq[:], yk[:, o, :], yk[:, o, :])
                nc.tensor.matmul(ps_sq[:], ones_s[:], ysq[:],
                                 start=(o == 0), stop=(o == 7))
            sqk = kpool.tile([1, cw], f32, tag="sqr")
            nc.scalar.activation(sqk[:], ps_sq[:], AF.Sqrt,
                                 bias=eps_s[:], scale=1.0 / DIM)
            rs = kpool.tile([1, cw], f32, tag="rs")
            nc.vector.reciprocal(rs[:], sqk[:])
            rsb = kpool.tile([128, cw], f32, tag="rsb")
            nc.gpsimd.partition_broadcast(rsb[:], rs[:])

            kr = kpool.tile([128, 8, cw], bf16, tag="kr")
            for o in range(8):
                ps_sw = pp.tile([128, cw], f32, tag="ps")
                nc.tensor.matmul(ps_sw[:], pt_s[:], yk[:, o, :])
                t1 = kpool.tile([128, cw], f32, tag="t1")
                nc.vector.tensor_mul(t1[:], yk[:, o, :], ctk_t[:])
                t2 = kpool.tile([128, cw], f32, tag="t2")
                nc.vector.tensor_mul(t2[:], ps_sw[:], stk_t[:])
                nc.vector.tensor_add(t1[:], t1[:], t2[:])
                nc.vector.tensor_mul(kr[:, o, :], t1[:], rsb[:])

            if _dbg and ci == 0:
                nc.sync.dma_start(ykdbg, yk[:])
                nc.sync.dma_start(rsbdbg, rsb[:])
                nc.sync.dma_start(krdbg, kr[:])
            v_sb = kpool.tile([128, ntt, DIM], bf16, tag="v")
            for tt in range(ntt):
                for oh in range(2):
                    ps_v = pp.tile([128, 512], f32, tag="ps")
                    for i in range(8):
                        nc.tensor.matmul(
                            ps_v[:], memt[:, i, tt * 128:(tt + 1) * 128],
                            wv_s[:, i, oh * 512:(oh + 1) * 512],
                            start=(i == 0), stop=(i == 7))
                    nc.vector.tensor_add(v_sb[:, tt, oh * 512:(oh + 1) * 512],
                                         ps_v[:], bv_s[:, oh * 512:(oh + 1) * 512])

            for h in range(8):
                pts = []
                for tt in range(ntt):
                    gtt = ct0 + tt
                    ps_s = pp_att.tile([128, SQ], f32, tag="psa")
                    nc.tensor.matmul(ps_s[:], kr[:, h, tt * 128:(tt + 1) * 128],
                                     qT[:, h, :])
                    pt = ppool.tile([128, SQ], bf16, tag="pt")
                    nc.scalar.activation(pt[:], ps_s[:], AF.Exp,
                                         bias=mb_s[:, gtt:gtt + 1], scale=SCALE)
                    pts.append(pt)
                    if gtt == 0:
                        nc.vector.tensor_copy(dacc[:, h, :], pt[:])
                    else:
                        nc.vector.tensor_add(dacc[:, h, :], dacc[:, h, :], pt[:])
                ps_n = pp_att.tile([128, SQ], f32, tag="psa")
                for tt in range(ntt):
                    nc.tensor.matmul(
                        ps_n[:], v_sb[:, tt, h * 128:(h + 1) * 128],
                        pts[tt][:], start=(tt == 0), stop=(tt == ntt - 1))
                if ci == 0:
                    nc.vector.tensor_copy(nacc[:, h, :], ps_n[:])
                else:
                    nc.vector.tensor_add(nacc[:, h, :], nacc[:, h, :], ps_n[:])
        ppool_cm.__exit__(None, None, None)
        kpool_cm.__exit__(None, None, None)

        # =========== reduce across cores ===========
        if _dbg:
            nc.sync.dma_start(dadbg, dacc[:])
            nc.sync.dma_start(nadbg, nacc[:])
        den = resid.tile([1, HEADS, SQ], f32)
        for h in range(8):
            ps_d = pp_sq.tile([1, SQ], f32, tag="pssq")
            nc.tensor.matmul(ps_d[:], ones_fs[:], dacc[:, h, :])
            nc.scalar.activation(den[0:1, h, :], ps_d[:], AF.Copy)
        nc.gpsimd.dma_start(
            cat[0:DIM, :].rearrange("(h p) q -> p h q", p=128), nacc[:])
        nc.gpsimd.dma_start(cat[DIM:DIM + HEADS, :], den[0:1, :, :])
        if _sim:
            nc.gpsimd.dma_start(cat_sh[:], cat[:])
        else:
            nc.gpsimd.collective_compute(
                "AllReduce", mybir.AluOpType.add,
                replica_groups=[list(range(N_CORES))],
                ins=[cat[:]], outs=[cat_sh[:]])

        if _dbg:
            nc.sync.dma_start(dendbg, den[:])
            nc.gpsimd.dma_start(catdbg, cat[:])
            nc.gpsimd.dma_start(catshdbg, cat_sh[:])
        # =========== per-core output projection on its query slice ===========
        tail = pool("tail", 1)
        wo_s = tail.tile([128, 8, 8, 128], bf16)
        nc.sync.dma_start(wo_s[:], wo)
        nred = tail.tile([128, 8, QS], f32)
        dred = tail.tile([1, HEADS, QS], f32)
        pid = nc.sync.partition_id()
        qoff = pid * QS
        nc.sync.dma_start(
            nred[:],
            cat_sh[0:DIM, bass.ds(qoff, QS)].rearrange("(h p) q -> p h q", p=128))
        nc.sync.dma_start(dred[:], cat_sh[DIM:DIM + HEADS, bass.ds(qoff, QS)])
        rd = tail.tile([1, HEADS, QS], f32)
        nc.vector.reciprocal(rd[:], dred[:])
        nsc = tail.tile([128, 8, QS], bf16)
        for h in range(8):
            rdb = tail.tile([128, QS], f32, tag="rdb")
            nc.gpsimd.partition_broadcast(rdb[:], rd[0:1, h, :])
            nc.vector.tensor_mul(nsc[:, h, :], nred[:, h, :], rdb[:])
        out_sb = tail.tile([128, 8, QS], f32)
        for e in range(8):
            ps_o = pp.tile([128, QS], f32, tag="ps")
            for o in range(8):
                nc.tensor.matmul(ps_o[:], wo_s[:, o, e, :], nsc[:, o, :],
                                 start=(o == 0), stop=(o == 7))
            nc.scalar.activation(out_sb[:, e, :], ps_o[:], AF.Identity,
                                 bias=bo_s[:, e:e + 1])
        nc.sync.dma_start(
            outT.rearrange("(e p) q -> p e q", p=128)[:, :, 0:QS], out_sb[:])

        for p in reversed(ctx_pools):
            p.__exit__(None, None, None)

    nc.compile()
    _cache["nc"] = nc
    return nc


def _prep(x, mem, mask, cos_q, sin_q, cos_k, sin_k,
          Wq, bq, Wk, bk, Wv, bv, Wo, bo, gq, gk):
    import ml_dtypes
    f = np.float32
    bf = ml_dtypes.bfloat16
    x = np.asarray(x, f).reshape(SQ, DIM)
    mem = np.asarray(mem, f).reshape(SK, DIM)
    mask = np.asarray(mask).reshape(SK)
    cos_q = np.asarray(cos_q, f)
    sin_q = np.asarray(sin_q, f)
    cos_k = np.asarray(cos_k, f)
    sin_k = np.asarray(sin_k, f)
    Wq, Wk, Wv, Wo = (np.asarray(w, f) for w in (Wq, Wk, Wv, Wo))
    bq, bk, bv, bo, gq, gk = (np.asarray(v, f) for v in (bq, bk, bv, bo, gq, gk))

    if not np.allclose(gk, 1.0):
        gkp = gk.reshape(-1, 2)
        assert np.allclose(gkp[:, 0], gkp[:, 1]), "unsupported non-pairwise gk"

    def tile_w(WT):  # [1024,1024] (in,out of W.T) -> [p, i, o, m]
        return np.ascontiguousarray(
            WT.reshape(8, 128, 8, 128).transpose(1, 0, 2, 3)).astype(bf)

    ii = np.arange(128)
    jj = ii // 2
    partner = ii ^ 1

    # fold gq (and pairwise gk) into the q rope tables; sin pairs with
    # partner's gq
    gq_t = (gq * gk).reshape(8, 128)
    gq_sin = (gq.reshape(8, 128)[:, partner] * gk.reshape(8, 128))
    cq = cos_q[:, jj].T                # [128, SQ]
    sq = sin_q[:, jj].T
    ctq = np.ascontiguousarray(
        (cq[None, :, :] * gq_t[:, :, None]).transpose(1, 0, 2)).astype(f)
    stq = np.ascontiguousarray(
        (sq[None, :, :] * gq_sin[:, :, None]).transpose(1, 0, 2)).astype(f)

    PT = np.zeros((128, 128), f)
    even = ii[ii % 2 == 0]
    PT[even + 1, even] = -1.0
    PT[even, even + 1] = 1.0

    shared = {
        "xT": np.ascontiguousarray(x.T).astype(bf),
        "wq": tile_w(Wq.T), "wk": tile_w(Wk.T), "wo": tile_w(Wo.T),
        "wv": np.ascontiguousarray(
            Wv.T.reshape(8, 128, DIM).transpose(1, 0, 2)).astype(bf),
        "ctq": ctq, "stq": stq,
        "bq_t": np.ascontiguousarray(bq.reshape(8, 128).T),
        "bk_t": np.ascontiguousarray(bk.reshape(8, 128).T),
        "bo_t": np.ascontiguousarray(bo.reshape(8, 128).T),
        "bv_t": np.ascontiguousarray(np.tile(bv, (128, 1))),
        "pmat": PT.astype(bf),
        "ones_c": np.ones((128, 1), bf),
        "ones_f": np.ones((128, 1), f),
        "eps_c": np.full((1, 1), EPS, f),
    }

    memT_full = np.zeros((DIM, N_CORES * SKC), bf)
    memT_full[:, :SK] = mem.T.astype(bf)
    ctk_full = np.zeros((HD, N_CORES * SKC), f)
    stk_full = np.zeros((HD, N_CORES * SKC), f)
    ctk_full[:, :SK] = cos_k[:, jj].T
    stk_full[:, :SK] = sin_k[:, jj].T
    mb_full = np.full(N_CORES * SKC, NEG, f)
    mb_full[:SK] = np.where(mask, 0.0, NEG)

    in_maps = []
    for c in range(N_CORES):
        s = slice(c * SKC, (c + 1) * SKC)
        m = dict(shared)
        m["memT"] = np.ascontiguousarray(memT_full[:, s])
        m["ctk"] = np.ascontiguousarray(ctk_full[:, s])
        m["stk"] = np.ascontiguousarray(stk_full[:, s])
        m["mbias"] = np.ascontiguousarray(mb_full[s].reshape(TT, 128).T)
        in_maps.append(m)
    return in_maps


def kernel(**inputs):
    from concourse.bass_utils import run_bass_kernel_spmd
    nc = _build()
    in_maps = _prep(**inputs)
    res = run_bass_kernel_spmd(nc, in_maps, list(range(N_CORES)))
    parts = [res.results[c]["outT"][:, 0:QS].T for c in range(N_CORES)]
    out = np.concatenate(parts, axis=0)
    return out[None].astype(np.float32)



# revision 15
# speedup vs baseline: 2.0202x; 2.0202x over previous
"""Trainium2 Bass kernel for nn_MemoryRetriever (cross-attention memory retriever).

Strategy (v2):
- Host-side mask compaction: masked-out keys (~50%) are dropped on the host;
  survivors are dealt evenly to the 8 cores (n_c ~ Nkeep/8 each, zero-padded
  to SKC keys/core, pads confined to each core's last 512-key chunk and
  neutralized by a -224 additive bias folded into the scores matmul).
- fp8e4 DoubleRow matmuls (0.5 cycles/row) for Q/K/V projections, scores,
  attention*V and the denominator reduction.  Weights are pre-scaled by 16 on
  the host (fp8e4 subnormal floor) and descaled on PSUM read-out.
- Scores carry the mask bias inside the same DoubleRow instruction: the
  stationary operand's second half holds the bias row (partition 0), the
  moving operand's second half is a one-hot row, so exp() needs no per-tile
  bias and fuses across tiles.
- Engine balance: PE does all matmuls; Act does exp + rms (ln/exp pair);
  DVE does rope muls; GPSIMD (Pool) does PSUM->SBUF descale copies and
  broadcasts; numerator/denominator PSUM banks are DMA'd straight to DRAM.
- One AllReduce of [1032, 512] fp32 partial numerators/denominators, then
  each core output-projects its own 64-query slice.
"""

import os
import sys
import numpy as np

sys.path.insert(0, "/opt/trn_rl_repo")

DIM = 1024
HEADS = 8
HD = 128
SQ = 512
N_CORES = 8
QS = SQ // N_CORES
EPS = 1e-6
SCALE = 1.0 / np.sqrt(128.0)
WS = 16.0            # host-side weight scale (fp8 subnormal avoidance)
SHIFT = -3.0         # exp(score + SHIFT): keeps fp8e4 pt in range
MBPAD = -224.0       # additive raw-score bias for padded keys (pre-SCALE)
CHT = 4              # key tiles (128) per chunk

_cache = {}


def _build(skc=2048):
    key = ("nc", skc)
    if key in _cache:
        return _cache[key]

    import concourse.bass as bass
    import concourse.tile as tile
    from concourse import mybir, bacc

    f32 = mybir.dt.float32
    bf16 = mybir.dt.bfloat16
    fp8 = mybir.dt.float8e4
    AF = mybir.ActivationFunctionType
    DR = mybir.MatmulPerfMode.DoubleRow

    TT = skc // 128          # key tiles per core
    NCH = TT // CHT          # chunks per core
    assert skc % (CHT * 128) == 0

    _sim = os.environ.get("KSIM", "0") == "1"

    nc = bacc.Bacc("TRN2", target_bir_lowering=False, debug=False,
                   num_devices=N_CORES)

    def din(name, shape, dt=f32):
        return nc.dram_tensor(name, list(shape), dt, kind="ExternalInput").ap()

    # per-core sharded inputs
    memT = din("memT", [DIM, skc], fp8)       # compacted mem shard, feature-major
    ctk = din("ctk", [HD, skc], bf16)         # K rope cos (pair-dup rows)
    stk = din("stk", [HD, skc], bf16)
    mbr = din("mbr", [1, HEADS, skc], fp8)    # 0 real / MBPAD pad, repeated per o
    # shared inputs
    xt = din("xt", [128, 8, SQ], fp8)         # x.T tiled [p,i,q]
    wq = din("wq", [128, 8, 8, 128], fp8)     # [p,i,o,m] = WS*Wq.T[i*128+p, o*128+m]
    wk = din("wk", [128, 8, 8, 128], fp8)
    wo = din("wo", [128, 8, 8, 128], fp8)
    wv = din("wv", [128, 8, DIM], fp8)        # [p,i,d] = WS*Wv.T[i*128+p, d]
    ctq = din("ctq", [128, 8, SQ], bf16)      # q rope cos (gq*gk folded)
    stq = din("stq", [128, 8, SQ], bf16)
    bq_t = din("bq_t", [128, 8])
    bk_t = din("bk_t", [128, 8])
    bo_t = din("bo_t", [128, 8])              # bo + Wo@bv folded
    pmat = din("pmat", [128, 128], bf16)      # P.T for rope pair swap (+-1)
    ones_c = din("ones_c", [128, 1], bf16)
    sel = din("sel", [128, 2, 64], fp8)       # den head selector
    qones = din("qones", [1, 8, SQ], fp8)     # one-hot rows for qT_dr half-1
    eps_c = din("eps_c", [1, 1])
    shf_c = din("shf_c", [128, 1])

    outT = nc.dram_tensor("outT", [DIM, SQ], f32, kind="ExternalOutput").ap()
    cat = nc.dram_tensor("cat", [DIM + HEADS, SQ], bf16)
    cat_sh = nc.dram_tensor("cat_sh", [DIM + HEADS, SQ], bf16,
                            addr_space="Shared")

    MUL = mybir.AluOpType.mult
    ADD = mybir.AluOpType.add

    with tile.TileContext(nc) as tc:
        ctx_pools = []   # list of (cm, entered)

        def pool(name, bufs, space=None):
            kw = dict(name=name, bufs=bufs)
            if space:
                kw["space"] = space
            cm = tc.tile_pool(**kw)
            entered = cm.__enter__()
            ctx_pools.append((cm, entered))
            return entered

        def close_pool(entered):
            for i, (cm, e) in enumerate(ctx_pools):
                if e is entered:
                    cm.__exit__(None, None, None)
                    ctx_pools.pop(i)
                    return
            raise KeyError("pool not found")

        consts = pool("consts", 1)
        resid = pool("resid", 1)
        pp_den = pool("pp_den", 1, space="PSUM")  # den [8,512]
        pp_s = pool("pp_s", 1, space="PSUM")      # swap + scores [128,2,512]
        pp_k = pool("pp_k", 2, space="PSUM")      # proj psum [128,512]
        pp_v = pool("pp_v", 1, space="PSUM")      # V psum [128,2,512]
        pp_sq = pool("pp_sq", 1, space="PSUM")    # sumsq [1,512]

        # ---- constants / resident tensors ----
        _cnt = [0]

        def cload(shape, dt, src, via=nc.sync):
            _cnt[0] += 1
            t = consts.tile(shape, dt, tag=f"c{_cnt[0]}")
            via.dma_start(t[:], src)
            return t

        pt_s = cload([128, 128], bf16, pmat)
        ones_s = cload([128, 1], bf16, ones_c)
        sel_s = cload([128, 2, 64], fp8, sel)
        bq_s = cload([128, 8], f32, bq_t)
        bk_s = cload([128, 8], f32, bk_t)
        bo_s = cload([128, 8], f32, bo_t)
        wq_s = cload([128, 8, 8, 128], fp8, wq)
        wk_s = cload([128, 8, 8, 128], fp8, wk)
        wv_s = cload([128, 8, DIM], fp8, wv)
        wo_s = cload([128, 8, 8, 128], fp8, wo)
        eps_s = cload([1, 1], f32, eps_c)
        shf_s = cload([128, 1], f32, shf_c)

        qT = resid.tile([128, 8, 2, SQ], fp8)       # [d, h, dr-half, q]
        kra = resid.tile([128, 8, CHT, 2, 128], fp8)
        krb = resid.tile([128, 8, CHT, 2, 128], fp8)
        pt_all = resid.tile([128, 8, TT, SQ], fp8)  # exp(scores+shift)
        v_sb = resid.tile([128, TT, DIM], fp8)
        yk = resid.tile([128, 8, 512], bf16)
        yq = yk

        # zero the DoubleRow second halves; one-hot row for qT
        nc.gpsimd.memset(qT[:, :, 1, :], 0.0)
        nc.sync.dma_start(qT[0:1, :, 1, :], qones)
        nc.gpsimd.memset(kra[:, :, :, 1, :], 0.0)
        nc.gpsimd.memset(krb[:, :, :, 1, :], 0.0)

        den_ps = pp_den.tile([8, SQ], f32)

        wpool = pool("wpool", 3)    # small working tiles (ysq/ykn/yc/ys)

        def rmsnorm_rope(ysrc, n, ct_of, st_of, out_half):
            """shared Q/K tail: sumsq -> rs -> rope -> fp8 out_half."""
            ps_sq = pp_sq.tile([1, n], f32, tag="pssq")
            for o in range(8):
                ysq = wpool.tile([128, n], bf16, tag="ysq")
                nc.vector.tensor_mul(ysq[:], ysrc[:, o, :], ysrc[:, o, :])
                nc.tensor.matmul(ps_sq[:], ones_s[:], ysq[:],
                                 start=(o == 0), stop=(o == 7))
            lnm = wpool.tile([1, n], f32, tag="lnm")
            nc.scalar.activation(lnm[:], ps_sq[:], AF.Ln,
                                 scale=1.0 / DIM, bias=eps_s[:])
            rs = wpool.tile([1, n], bf16, tag="rs")
            nc.scalar.activation(rs[:], lnm[:], AF.Exp, scale=-0.5)
            rsb = wpool.tile([128, n], bf16, tag="rsb")
            nc.gpsimd.partition_broadcast(rsb[:], rs[:])
            swp = pp_s.tile([128, 2, n], f32, tag="ps_s")
            for o in range(8):
                ykn = wpool.tile([128, n], bf16, tag="ykn")
                nc.vector.tensor_mul(ykn[:], ysrc[:, o, :], rsb[:])
                ys = wpool.tile([128, n], bf16, tag="ys")
                nc.vector.tensor_mul(ys[:], ykn[:], st_of(o))
                nc.tensor.matmul(swp[:, o % 2, :], pt_s[:], ys[:])
                yc = wpool.tile([128, n], bf16, tag="yc")
                nc.vector.tensor_mul(yc[:], ykn[:], ct_of(o))
                nc.vector.tensor_add(out_half(o), yc[:], swp[:, o % 2, :])

        # =========== Q phase ===========
        qpool = pool("qpool", 1)
        ctq_s = qpool.tile([128, 8, SQ], bf16, tag="ctq")
        nc.gpsimd.dma_start(ctq_s[:], ctq)
        stq_s = qpool.tile([128, 8, SQ], bf16, tag="stq")
        nc.gpsimd.dma_start(stq_s[:], stq)
        xt_s = qpool.tile([128, 8, SQ], fp8, tag="xt")
        nc.gpsimd.dma_start(xt_s[:], xt)
        for o in range(8):
            ps_q = pp_k.tile([128, SQ], f32, tag="ps_k")
            for pr in range(4):
                nc.tensor.matmul(ps_q[:], wq_s[:, 2 * pr:2 * pr + 2, o, :],
                                 xt_s[:, 2 * pr:2 * pr + 2, :],
                                 start=(pr == 0), stop=(pr == 3), perf_mode=DR)
            nc.gpsimd.tensor_scalar(yq[:, o, :], ps_q[:], 1.0 / WS,
                                    bq_s[:, o:o + 1], MUL, ADD)
        rmsnorm_rope(yq, SQ, lambda o: ctq_s[:, o, :], lambda o: stq_s[:, o, :],
                     lambda o: qT[:, o, 0, :])
        close_pool(qpool)

        # =========== chunk loop: proj + rope + scores + exp + den ==========
        kpool = pool("kpool", 2)
        for c in range(NCH):
            kr = kra if c % 2 == 0 else krb
            c0 = c * CHT * 128
            cw = CHT * 128
            memt = kpool.tile([128, 8, cw], fp8, tag="memt")
            nc.gpsimd.dma_start(
                memt[:], memT[:, c0:c0 + cw].rearrange("(i p) t -> p i t", p=128))
            ctk_t = kpool.tile([128, cw], bf16, tag="ctk")
            nc.gpsimd.dma_start(ctk_t[:], ctk[:, c0:c0 + cw])
            stk_t = kpool.tile([128, cw], bf16, tag="stk")
            nc.gpsimd.dma_start(stk_t[:], stk[:, c0:c0 + cw])
            nc.gpsimd.dma_start(
                kr[0:1, :, :, 1, :],
                mbr[0:1, :, c0:c0 + cw].rearrange("a o (t m) -> a o t m", m=128))

            # K proj (+ V proj interleaved on odd o)
            for o in range(8):
                ps_k = pp_k.tile([128, cw], f32, tag="ps_k")
                for pr in range(4):
                    nc.tensor.matmul(ps_k[:], wk_s[:, 2 * pr:2 * pr + 2, o, :],
                                     memt[:, 2 * pr:2 * pr + 2, :],
                                     start=(pr == 0), stop=(pr == 3),
                                     perf_mode=DR)
                nc.gpsimd.tensor_scalar(yk[:, o, :], ps_k[:], 1.0 / WS,
                                        bk_s[:, o:o + 1], MUL, ADD)
                if o % 2 == 1:
                    t = o // 2
                    ps_v = pp_v.tile([128, 2, 512], f32, tag="ps_v")
                    for oh in range(2):
                        for pr in range(4):
                            nc.tensor.matmul(
                                ps_v[:, oh, :],
                                memt[:, 2 * pr:2 * pr + 2, t * 128:(t + 1) * 128],
                                wv_s[:, 2 * pr:2 * pr + 2,
                                     oh * 512:(oh + 1) * 512],
                                start=(pr == 0), stop=(pr == 3), perf_mode=DR)
                    nc.gpsimd.tensor_scalar(v_sb[:, c * CHT + t, :], ps_v[:],
                                            1.0 / WS, 0.0, MUL, ADD)

            rmsnorm_rope(yk, cw, lambda o: ctk_t[:], lambda o: stk_t[:],
                         lambda o: kr[:, o, :, 0, :])

            # scores + exp + den
            for p2 in range(2):
                for h in range(8):
                    ps_s = pp_s.tile([128, 2, 512], f32, tag="ps_s")
                    for tt in range(2):
                        nc.tensor.matmul(ps_s[:, tt, :],
                                         kr[:, h, p2 * 2 + tt, :, :],
                                         qT[:, h, :, :], perf_mode=DR)
                    gp = c * 2 + p2
                    ptt = pt_all[:, h, gp * 2:gp * 2 + 2, :]
                    nc.scalar.activation(ptt, ps_s[:], AF.Exp,
                                         scale=SCALE, bias=shf_s[:])
                    nc.tensor.matmul(den_ps[:], sel_s[:, :, h * 8:h * 8 + 8],
                                     ptt, perf_mode=DR,
                                     start=(c == 0 and p2 == 0 and h == 0),
                                     stop=(c == NCH - 1 and p2 == 1 and h == 7))

        for p in (kpool, wpool, pp_sq, pp_v, pp_k):
            close_pool(p)
        close_pool(pp_s)

        # =========== numerator (attn @ V) ===========
        pp_n = pool("pp_n", 2, space="PSUM")
        nacc = resid.tile([128, 8, SQ], bf16)
        dacc = resid.tile([8, SQ], bf16)
        for h in range(8):
            ps_n = pp_n.tile([128, SQ], f32, tag="ps_n")
            for p in range(TT // 2):
                nc.tensor.matmul(ps_n[:],
                                 v_sb[:, 2 * p:2 * p + 2, h * 128:(h + 1) * 128],
                                 pt_all[:, h, 2 * p:2 * p + 2, :],
                                 start=(p == 0), stop=(p == TT // 2 - 1),
                                 perf_mode=DR)
            nc.scalar.activation(nacc[:, h, :], ps_n[:], AF.Copy)
        nc.scalar.activation(dacc[:], den_ps[:], AF.Copy)
        nc.gpsimd.dma_start(
            cat[0:DIM, :].rearrange("(h p) q -> p h q", p=128), nacc[:])
        nc.gpsimd.dma_start(cat[DIM:DIM + HEADS, :], dacc[:])

        # =========== reduce across cores ===========
        if _sim:
            nc.gpsimd.dma_start(cat_sh[:], cat[:])
        else:
            nc.gpsimd.collective_compute(
                "AllReduce", mybir.AluOpType.add,
                replica_groups=[list(range(N_CORES))],
                ins=[cat[:]], outs=[cat_sh[:]])

        # =========== per-core output projection on its query slice ==========
        tail = pool("tail", 1)
        nred = tail.tile([128, 8, QS], bf16)
        dred = tail.tile([1, HEADS, QS], bf16)
        pid = nc.sync.partition_id()
        qoff = pid * QS
        nc.sync.dma_start(
            nred[:],
            cat_sh[0:DIM, bass.ds(qoff, QS)].rearrange("(h p) q -> p h q", p=128))
        nc.sync.dma_start(dred[:], cat_sh[DIM:DIM + HEADS, bass.ds(qoff, QS)])
        rd = tail.tile([1, HEADS, QS], f32)
        nc.vector.reciprocal(rd[:], dred[:])
        nsc = tail.tile([128, 8, QS], fp8)
        for h in range(8):
            rdb = tail.tile([128, QS], f32, tag="rdb")
            nc.gpsimd.partition_broadcast(rdb[:], rd[0:1, h, :])
            nc.vector.tensor_mul(nsc[:, h, :], nred[:, h, :], rdb[:])
        out_sb = tail.tile([128, 8, QS], f32)
        for e in range(8):
            ps_o = pp_n.tile([128, QS], f32, tag="ps_n")
            for pr in range(4):
                nc.tensor.matmul(ps_o[:], wo_s[:, 2 * pr:2 * pr + 2, e, :],
                                 nsc[:, 2 * pr:2 * pr + 2, :],
                                 start=(pr == 0), stop=(pr == 3), perf_mode=DR)
            nc.scalar.activation(out_sb[:, e, :], ps_o[:], AF.Identity,
                                 scale=1.0 / WS, bias=bo_s[:, e:e + 1])
        nc.sync.dma_start(
            outT.rearrange("(e p) q -> p e q", p=128)[:, :, 0:QS], out_sb[:])

        for cm, _ in reversed(ctx_pools):
            cm.__exit__(None, None, None)

    nc.compile()
    _cache[key] = nc
    _cache["nc"] = nc
    return nc


def _skc_for(nkeep):
    return max(CHT * 128, int(np.ceil(nkeep / (N_CORES * 512))) * 512)


def _prep(x, mem, mask, cos_q, sin_q, cos_k, sin_k,
          Wq, bq, Wk, bk, Wv, bv, Wo, bo, gq, gk):
    import ml_dtypes
    f = np.float32
    bf = ml_dtypes.bfloat16
    f8 = ml_dtypes.float8_e4m3
    x = np.asarray(x, f).reshape(SQ, DIM)
    mem = np.asarray(mem, f).reshape(-1, DIM)
    mask = np.asarray(mask).reshape(-1)
    cos_q = np.asarray(cos_q, f)
    sin_q = np.asarray(sin_q, f)
    cos_k = np.asarray(cos_k, f)
    sin_k = np.asarray(sin_k, f)
    Wq, Wk, Wv, Wo = (np.asarray(w, f) for w in (Wq, Wk, Wv, Wo))
    bq, bk, bv, bo, gq, gk = (np.asarray(v, f) for v in (bq, bk, bv, bo, gq, gk))

    if not np.allclose(gk, 1.0):
        gkp = gk.reshape(-1, 2)
        assert np.allclose(gkp[:, 0], gkp[:, 1]), "unsupported non-pairwise gk"

    idx = np.flatnonzero(mask)
    nkeep = len(idx)
    skc = _skc_for(nkeep)
    base, rem = divmod(nkeep, N_CORES)
    counts = [base + (1 if c < rem else 0) for c in range(N_CORES)]
    offs = np.concatenate([[0], np.cumsum(counts)])

    def tile_w(WT):  # [1024,1024] (in,out of W.T) -> [p, i, o, m], scaled
        return np.ascontiguousarray(
            (WT * WS).reshape(8, 128, 8, 128).transpose(1, 0, 2, 3)).astype(f8)

    ii = np.arange(128)
    jj = ii // 2
    partner = ii ^ 1

    # fold gq (and pairwise gk) into the q rope tables; sin pairs with
    # partner's gq
    gq_t = (gq * gk).reshape(8, 128)
    gq_sin = (gq.reshape(8, 128)[:, partner] * gk.reshape(8, 128))
    cq = cos_q[:, jj].T                # [128, SQ]
    sq = sin_q[:, jj].T
    ctq = np.ascontiguousarray(
        (cq[None, :, :] * gq_t[:, :, None]).transpose(1, 0, 2)).astype(bf)
    stq = np.ascontiguousarray(
        (sq[None, :, :] * gq_sin[:, :, None]).transpose(1, 0, 2)).astype(bf)

    PT = np.zeros((128, 128), f)
    even = ii[ii % 2 == 0]
    PT[even + 1, even] = -1.0
    PT[even, even + 1] = 1.0

    selm = np.zeros((128, 2, 64), f)
    for h in range(8):
        selm[:, :, h * 8 + h] = 1.0

    # one-hot row lives on partition 0 only; this input IS partition 0's row
    qones = np.ones((1, 8, SQ), f).astype(f8)

    bo_f = bo + Wo @ bv

    shared = {
        "xt": np.ascontiguousarray(
            x.T.reshape(8, 128, SQ).transpose(1, 0, 2)).astype(f8),
        "wq": tile_w(Wq.T), "wk": tile_w(Wk.T), "wo": tile_w(Wo.T),
        "wv": np.ascontiguousarray(
            (Wv.T * WS).reshape(8, 128, DIM).transpose(1, 0, 2)).astype(f8),
        "ctq": ctq, "stq": stq,
        "bq_t": np.ascontiguousarray(bq.reshape(8, 128).T),
        "bk_t": np.ascontiguousarray(bk.reshape(8, 128).T),
        "bo_t": np.ascontiguousarray(bo_f.reshape(8, 128).T),
        "pmat": PT.astype(bf),
        "ones_c": np.ones((128, 1), bf),
        "sel": selm.astype(f8),
        "qones": qones,
        "eps_c": np.full((1, 1), EPS, f),
        "shf_c": np.full((128, 1), SHIFT, f),
    }

    ckT = cos_k[:, jj].T.astype(f)     # [128, SK]
    skT = sin_k[:, jj].T.astype(f)

    in_maps = []
    for c in range(N_CORES):
        keys = idx[offs[c]:offs[c + 1]]
        n = len(keys)
        m = dict(shared)
        memc = np.zeros((DIM, skc), f8)
        memc[:, :n] = mem[keys].T.astype(f8)
        ctkc = np.zeros((HD, skc), bf)
        stkc = np.zeros((HD, skc), bf)
        ctkc[:, :n] = ckT[:, keys].astype(bf)
        stkc[:, :n] = skT[:, keys].astype(bf)
        mb = np.full((skc,), MBPAD, f)
        mb[:n] = 0.0
        m["memT"] = memc
        m["ctk"] = np.ascontiguousarray(ctkc)
        m["stk"] = np.ascontiguousarray(stkc)
        m["mbr"] = np.ascontiguousarray(
            np.broadcast_to(mb[None, None, :], (1, HEADS, skc))).astype(f8)
        in_maps.append(m)
    return in_maps


def _assemble(outTs):
    parts = [np.asarray(outTs[c])[:, 0:QS].T for c in range(N_CORES)]
    out = np.concatenate(parts, axis=0)
    return out[None].astype(np.float32)


def kernel(**inputs):
    from concourse.bass_utils import run_bass_kernel_spmd
    in_maps = _prep(**inputs)
    skc = in_maps[0]["memT"].shape[1]
    nc = _build(skc)
    res = run_bass_kernel_spmd(nc, in_maps, list(range(N_CORES)))
    return _assemble([res.results[c]["outT"] for c in range(N_CORES)])


# revision 16
# speedup vs baseline: 2.7944x; 1.3833x over previous
"""Trainium2 Bass kernel for nn_MemoryRetriever (cross-attention memory retriever).

Strategy (v2):
- Host-side mask compaction: masked-out keys (~50%) are dropped on the host;
  survivors are dealt evenly to the 8 cores (n_c ~ Nkeep/8 each, zero-padded
  to SKC keys/core, pads confined to each core's last 512-key chunk and
  neutralized by a -224 additive bias folded into the scores matmul).
- fp8e4 DoubleRow matmuls (0.5 cycles/row) for Q/K/V projections, scores,
  attention*V and the denominator reduction.  Weights are pre-scaled by 16 on
  the host (fp8e4 subnormal floor) and descaled on PSUM read-out.
- Scores carry the mask bias inside the same DoubleRow instruction: the
  stationary operand's second half holds the bias row (partition 0), the
  moving operand's second half is a one-hot row, so exp() needs no per-tile
  bias and fuses across tiles.
- Engine balance: PE does all matmuls; Act does exp + rms (ln/exp pair);
  DVE does rope muls; GPSIMD (Pool) does PSUM->SBUF descale copies and
  broadcasts; numerator/denominator PSUM banks are DMA'd straight to DRAM.
- One AllReduce of [1032, 512] fp32 partial numerators/denominators, then
  each core output-projects its own 64-query slice.
"""

import os
import sys
import numpy as np

sys.path.insert(0, "/opt/trn_rl_repo")

DIM = 1024
HEADS = 8
HD = 128
SQ = 512
N_CORES = 8
QS = SQ // N_CORES
EPS = 1e-6
SCALE = 1.0 / np.sqrt(128.0)
WS = 16.0            # host-side weight scale (fp8 subnormal avoidance)
SHIFT = -3.0         # exp(score + SHIFT): keeps fp8e4 pt in range
MBPAD = -224.0       # additive raw-score bias for padded keys (pre-SCALE)
CHT = 4              # key tiles (128) per chunk

_cache = {}


def _build(skc=2048):
    key = ("nc", skc)
    if key in _cache:
        return _cache[key]

    import concourse.bass as bass
    import concourse.tile as tile
    from concourse import mybir, bacc

    f32 = mybir.dt.float32
    bf16 = mybir.dt.bfloat16
    fp8 = mybir.dt.float8e4
    AF = mybir.ActivationFunctionType
    DR = mybir.MatmulPerfMode.DoubleRow

    TT = skc // 128          # key tiles per core
    NCH = TT // CHT          # chunks per core
    assert skc % (CHT * 128) == 0

    _sim = os.environ.get("KSIM", "0") == "1"

    nc = bacc.Bacc("TRN2", target_bir_lowering=False, debug=False,
                   num_devices=N_CORES)

    def din(name, shape, dt=f32):
        return nc.dram_tensor(name, list(shape), dt, kind="ExternalInput").ap()

    # per-core sharded inputs
    memT = din("memT", [DIM, skc], fp8)       # compacted mem shard, feature-major
    ctk = din("ctk", [HD, skc], bf16)         # K rope cos (pair-dup rows)
    stk = din("stk", [HD, skc], bf16)
    mbr = din("mbr", [1, HEADS, skc], fp8)    # 0 real / MBPAD pad, repeated per o
    # shared inputs
    xt = din("xt", [128, 8, SQ], fp8)         # x.T tiled [p,i,q]
    wq = din("wq", [128, 8, 8, 128], fp8)     # [p,i,o,m] = WS*Wq.T[i*128+p, o*128+m]
    wk = din("wk", [128, 8, 8, 128], fp8)
    wo = din("wo", [128, 8, 8, 128], fp8)
    wv = din("wv", [128, 8, DIM], fp8)        # [p,i,d] = WS*Wv.T[i*128+p, d]
    ctq = din("ctq", [128, 8, SQ], bf16)      # q rope cos (gq*gk folded)
    stq = din("stq", [128, 8, SQ], bf16)
    bq_t = din("bq_t", [128, 8])
    bk_t = din("bk_t", [128, 8])
    bo_t = din("bo_t", [128, 8])              # bo + Wo@bv folded
    pmat = din("pmat", [128, 128], bf16)      # P.T for rope pair swap (+-1)
    ones_c = din("ones_c", [128, 1], bf16)
    sel = din("sel", [128, 2, 64], fp8)       # den head selector
    qones = din("qones", [1, 8, SQ], fp8)     # one-hot rows for qT_dr half-1
    eps_c = din("eps_c", [1, 1])
    shf_c = din("shf_c", [128, 1])

    outT = nc.dram_tensor("outT", [DIM, SQ], f32, kind="ExternalOutput").ap()
    cat = nc.dram_tensor("cat", [DIM + HEADS, SQ], bf16)
    cat_sh = nc.dram_tensor("cat_sh", [DIM + HEADS, SQ], bf16,
                            addr_space="Shared")

    MUL = mybir.AluOpType.mult
    ADD = mybir.AluOpType.add

    with tile.TileContext(nc) as tc:
        ctx_pools = []   # list of (cm, entered)

        def pool(name, bufs, space=None):
            kw = dict(name=name, bufs=bufs)
            if space:
                kw["space"] = space
            cm = tc.tile_pool(**kw)
            entered = cm.__enter__()
            ctx_pools.append((cm, entered))
            return entered

        def close_pool(entered):
            for i, (cm, e) in enumerate(ctx_pools):
                if e is entered:
                    cm.__exit__(None, None, None)
                    ctx_pools.pop(i)
                    return
            raise KeyError("pool not found")

        consts = pool("consts", 1)
        resid = pool("resid", 1)
        pp_den = pool("pp_den", 1, space="PSUM")  # den [8,512]
        pp_s = pool("pp_s", 1, space="PSUM")      # swap + scores [128,2,512]

        # ---- constants / resident tensors ----
        _cnt = [0]

        def cload(shape, dt, src, via=nc.sync):
            _cnt[0] += 1
            t = consts.tile(shape, dt, tag=f"c{_cnt[0]}")
            via.dma_start(t[:], src)
            return t

        pt_s = cload([128, 128], bf16, pmat)
        ones_s = cload([128, 1], bf16, ones_c)
        sel_s = cload([128, 2, 64], fp8, sel)
        bq_s = cload([128, 8], f32, bq_t)
        bk_s = cload([128, 8], f32, bk_t)
        bo_s = cload([128, 8], f32, bo_t)
        wq_s = cload([128, 8, 8, 128], fp8, wq)
        wk_s = cload([128, 8, 8, 128], fp8, wk)
        wv_s = cload([128, 8, DIM], fp8, wv)
        wo_s = cload([128, 8, 8, 128], fp8, wo)
        eps_s = cload([1, 1], f32, eps_c)
        shf_s = cload([128, 1], f32, shf_c)

        qT = resid.tile([128, 8, 2, SQ], fp8)       # [d, h, dr-half, q]
        kra = resid.tile([128, 8, CHT, 2, 128], fp8)
        krb = resid.tile([128, 8, CHT, 2, 128], fp8)
        pt_all = resid.tile([128, 8, TT, SQ], fp8)  # exp(scores+shift)
        v_sb = resid.tile([128, TT, DIM], fp8)
        yk = resid.tile([128, 8, 512], bf16)
        yq = yk

        # zero the DoubleRow second halves; one-hot row for qT
        nc.gpsimd.memset(qT[:, :, 1, :], 0.0)
        nc.sync.dma_start(qT[0:1, :, 1, :], qones)
        nc.gpsimd.memset(kra[:, :, :, 1, :], 0.0)
        nc.gpsimd.memset(krb[:, :, :, 1, :], 0.0)

        den_ps = pp_den.tile([8, SQ], f32)
        nacc = resid.tile([128, 8, SQ], bf16)
        dacc = resid.tile([8, SQ], bf16)

        wpool = pool("wpool", 2)    # small working tiles (ysq/ykn/yc/ys)
        pp_all = pool("pp_all", 2, space="PSUM")   # [128,512] proj/V/swap psum
        pp_sq2 = pool("pp_sq2", 1, space="PSUM")   # sumsq [1,512]
        kpool = pool("kpool", 2)

        POW = mybir.AluOpType.pow

        def rs_broadcast(ps_sq, n):
            """rs = (sumsq/DIM + eps)^-0.5 via GPSIMD pow, then broadcast."""
            t = wpool.tile([1, n], f32, tag="lnm")
            nc.gpsimd.tensor_scalar(t[:], ps_sq[:], 1.0 / DIM, EPS, MUL, ADD)
            rs = wpool.tile([1, n], bf16, tag="rs")
            nc.gpsimd.tensor_scalar(rs[:], t[:], -0.5, 1.0, POW, MUL)
            rsb = wpool.tile([128, n], bf16, tag="rsb")
            nc.gpsimd.partition_broadcast(rsb[:], rs[:])
            return rsb

        def rope(ysrc, n, rsb, ct_of, st_of, out_half):
            for o in range(8):
                ykn = wpool.tile([128, n], bf16, tag="ykn")
                nc.vector.tensor_mul(ykn[:], ysrc[:, o, :], rsb[:])
                ys = wpool.tile([128, n], bf16, tag="ys")
                nc.vector.tensor_mul(ys[:], ykn[:], st_of(o))
                swp = pp_all.tile([128, n], f32, tag="pp")
                nc.tensor.matmul(swp[:], pt_s[:], ys[:])
                yc = wpool.tile([128, n], bf16, tag="yc")
                nc.vector.tensor_mul(yc[:], ykn[:], ct_of(o))
                nc.vector.tensor_add(out_half(o), yc[:], swp[:])

        def unit_K(w_s, b_s, src, ydst, o, ps_sq):
            """one output-block projection + descale + sumsq contribution."""
            ps = pp_all.tile([128, 512], f32, tag="pp")
            for pr in range(4):
                nc.tensor.matmul(ps[:], w_s[:, 2 * pr:2 * pr + 2, o, :],
                                 src[:, 2 * pr:2 * pr + 2, :],
                                 start=(pr == 0), stop=(pr == 3), perf_mode=DR)
            nc.gpsimd.tensor_scalar(ydst[:, o, :], ps[:], 1.0 / WS,
                                    b_s[:, o:o + 1], MUL, ADD)
            ysq = wpool.tile([128, 512], bf16, tag="ysq")
            nc.vector.tensor_mul(ysq[:], ydst[:, o, :], ydst[:, o, :])
            nc.tensor.matmul(ps_sq[:], ones_s[:], ysq[:],
                             start=(o == 0), stop=(o == 7))

        def unit_V(memt, gt, t):
            """V projection for key tile t of the current chunk."""
            for oh in range(2):
                ps = pp_all.tile([128, 512], f32, tag="pp")
                for pr in range(4):
                    nc.tensor.matmul(
                        ps[:],
                        memt[:, 2 * pr:2 * pr + 2, t * 128:(t + 1) * 128],
                        wv_s[:, 2 * pr:2 * pr + 2, oh * 512:(oh + 1) * 512],
                        start=(pr == 0), stop=(pr == 3), perf_mode=DR)
                nc.scalar.activation(v_sb[:, gt, oh * 512:(oh + 1) * 512],
                                     ps[:], AF.Identity, scale=1.0 / WS)

        def group_SE(c, kr, h, p2):
            """scores + exp for (head h, tile-pair p2) of chunk c."""
            ps_s = pp_s.tile([128, 2, 512], f32, tag="ps_s")
            for tt in range(2):
                nc.tensor.matmul(ps_s[:, tt, :], kr[:, h, p2 * 2 + tt, :, :],
                                 qT[:, h, :, :], perf_mode=DR)
            gp = c * 2 + p2
            ptt = pt_all[:, h, gp * 2:gp * 2 + 2, :]
            nc.scalar.activation(ptt, ps_s[:], AF.Exp,
                                 scale=SCALE, bias=shf_s[:])

        def den_mm(c, h, p2):
            gp = c * 2 + p2
            nc.tensor.matmul(den_ps[:], sel_s[:, :, h * 8:h * 8 + 8],
                             pt_all[:, h, gp * 2:gp * 2 + 2, :], perf_mode=DR,
                             start=(c == 0 and p2 == 0 and h == 0),
                             stop=(c == NCH - 1 and p2 == 1 and h == 7))

        def numer(h, pp_n):
            ps_n = pp_n.tile([128, SQ], f32, tag="ps_n")
            for p in range(TT // 2):
                nc.tensor.matmul(ps_n[:],
                                 v_sb[:, 2 * p:2 * p + 2, h * 128:(h + 1) * 128],
                                 pt_all[:, h, 2 * p:2 * p + 2, :],
                                 start=(p == 0), stop=(p == TT // 2 - 1),
                                 perf_mode=DR)
            nc.vector.tensor_copy(nacc[:, h, :], ps_n[:])

        # =========== Q phase ===========
        qpool = pool("qpool", 1)
        ctq_s = qpool.tile([128, 8, SQ], bf16, tag="ctq")
        nc.sync.dma_start(ctq_s[:], ctq)
        stq_s = qpool.tile([128, 8, SQ], bf16, tag="stq")
        nc.sync.dma_start(stq_s[:], stq)
        xt_s = qpool.tile([128, 8, SQ], fp8, tag="xt")
        nc.sync.dma_start(xt_s[:], xt)
        ps_sqq = pp_sq2.tile([1, SQ], f32, tag="pssq")
        for o in range(8):
            unit_K(wq_s, bq_s, xt_s, yq, o, ps_sqq)
        rsb_q = rs_broadcast(ps_sqq, SQ)
        rope(yq, SQ, rsb_q, lambda o: ctq_s[:, o, :], lambda o: stq_s[:, o, :],
             lambda o: qT[:, o, 0, :])
        close_pool(qpool)

        # =========== pipelined chunk loop ===========
        cw = CHT * 128
        st = {}

        def s1_load(c):
            c0 = c * cw
            memt = kpool.tile([128, 8, cw], fp8, tag="memt")
            nc.sync.dma_start(
                memt[:], memT[:, c0:c0 + cw].rearrange("(i p) t -> p i t", p=128))
            ctk_t = kpool.tile([128, cw], bf16, tag="ctk")
            nc.sync.dma_start(ctk_t[:], ctk[:, c0:c0 + cw])
            stk_t = kpool.tile([128, cw], bf16, tag="stk")
            nc.sync.dma_start(stk_t[:], stk[:, c0:c0 + cw])
            kr = kra if c % 2 == 0 else krb
            nc.sync.dma_start(
                kr[0:1, :, :, 1, :],
                mbr[0:1, :, c0:c0 + cw].rearrange("a o (t m) -> a o t m", m=128))
            return dict(memt=memt, ctk=ctk_t, stk=stk_t, kr=kr)

        pp_n = None
        for it in range(NCH + 1):
            last = it == NCH
            if not last:
                st[it] = s1_load(it)
                ps_sq = pp_sq2.tile([1, cw], f32, tag="pssq")
                units = []
                for o in range(8):
                    units.append(lambda o=o: unit_K(wk_s, bk_s, st[it]["memt"],
                                                    yk, o, ps_sq))
                    if o % 2 == 1:
                        units.append(lambda o=o, c=it: unit_V(
                            st[c]["memt"], c * CHT + o // 2, o // 2))
            else:
                # tail iteration: close proj pools, open numerator psum
                for p in (kpool, pp_sq2, pp_all, wpool):
                    close_pool(p)
                pp_n = pool("pp_n", 2, space="PSUM")
                units = []

            groups = []
            if it >= 1:
                ckr = kra if (it - 1) % 2 == 0 else krb
                for h in range(8):
                    for p2 in range(2):
                        groups.append(
                            lambda h=h, p2=p2, c=it - 1, kr=ckr:
                            group_SE(c, kr, h, p2))
                    if last:
                        groups.append(lambda h=h, c=it - 1: (
                            den_mm(c, h, 0), den_mm(c, h, 1), numer(h, pp_n)))

            # interleave: spread score groups among proj units
            nu, ng = len(units), len(groups)
            gi = 0
            for ui, u in enumerate(units):
                u()
                want = (ui + 1) * ng // max(nu, 1)
                while gi < want:
                    groups[gi]()
                    gi += 1
            while gi < ng:
                groups[gi]()
                gi += 1

            if it >= 1 and not last:
                for h in range(8):
                    for p2 in range(2):
                        den_mm(it - 1, h, p2)
            if not last:
                rsb = rs_broadcast(ps_sq, cw)
                kr = st[it]["kr"]
                rope(yk, cw, rsb,
                     lambda o: st[it]["ctk"][:],
                     lambda o: st[it]["stk"][:],
                     lambda o, kr=kr: kr[:, o, :, 0, :])

        nc.scalar.activation(dacc[:], den_ps[:], AF.Copy)
        nc.gpsimd.dma_start(
            cat[0:DIM, :].rearrange("(h p) q -> p h q", p=128), nacc[:])
        nc.gpsimd.dma_start(cat[DIM:DIM + HEADS, :], dacc[:])

        # =========== reduce across cores ===========
        if _sim:
            nc.gpsimd.dma_start(cat_sh[:], cat[:])
        else:
            nc.gpsimd.collective_compute(
                "AllReduce", mybir.AluOpType.add,
                replica_groups=[list(range(N_CORES))],
                ins=[cat[:]], outs=[cat_sh[:]])

        # =========== per-core output projection on its query slice ==========
        tail = pool("tail", 1)
        nred = tail.tile([128, 8, QS], bf16)
        dred = tail.tile([1, HEADS, QS], bf16)
        pid = nc.sync.partition_id()
        qoff = pid * QS
        nc.sync.dma_start(
            nred[:],
            cat_sh[0:DIM, bass.ds(qoff, QS)].rearrange("(h p) q -> p h q", p=128))
        nc.sync.dma_start(dred[:], cat_sh[DIM:DIM + HEADS, bass.ds(qoff, QS)])
        rd = tail.tile([1, HEADS, QS], f32)
        nc.vector.reciprocal(rd[:], dred[:])
        nsc = tail.tile([128, 8, QS], fp8)
        for h in range(8):
            rdb = tail.tile([128, QS], f32, tag="rdb")
            nc.gpsimd.partition_broadcast(rdb[:], rd[0:1, h, :])
            nc.vector.tensor_mul(nsc[:, h, :], nred[:, h, :], rdb[:])
        out_sb = tail.tile([128, 8, QS], f32)
        for e in range(8):
            ps_o = pp_n.tile([128, QS], f32, tag="ps_n")
            for pr in range(4):
                nc.tensor.matmul(ps_o[:], wo_s[:, 2 * pr:2 * pr + 2, e, :],
                                 nsc[:, 2 * pr:2 * pr + 2, :],
                                 start=(pr == 0), stop=(pr == 3), perf_mode=DR)
            nc.scalar.activation(out_sb[:, e, :], ps_o[:], AF.Identity,
                                 scale=1.0 / WS, bias=bo_s[:, e:e + 1])
        nc.sync.dma_start(
            outT.rearrange("(e p) q -> p e q", p=128)[:, :, 0:QS], out_sb[:])

        for cm, _ in reversed(ctx_pools):
            cm.__exit__(None, None, None)

    nc.compile()
    _cache[key] = nc
    _cache["nc"] = nc
    return nc


def _skc_for(nkeep):
    return max(CHT * 128, int(np.ceil(nkeep / (N_CORES * 512))) * 512)


def _prep(x, mem, mask, cos_q, sin_q, cos_k, sin_k,
          Wq, bq, Wk, bk, Wv, bv, Wo, bo, gq, gk):
    import ml_dtypes
    f = np.float32
    bf = ml_dtypes.bfloat16
    f8 = ml_dtypes.float8_e4m3
    x = np.asarray(x, f).reshape(SQ, DIM)
    mem = np.asarray(mem, f).reshape(-1, DIM)
    mask = np.asarray(mask).reshape(-1)
    cos_q = np.asarray(cos_q, f)
    sin_q = np.asarray(sin_q, f)
    cos_k = np.asarray(cos_k, f)
    sin_k = np.asarray(sin_k, f)
    Wq, Wk, Wv, Wo = (np.asarray(w, f) for w in (Wq, Wk, Wv, Wo))
    bq, bk, bv, bo, gq, gk = (np.asarray(v, f) for v in (bq, bk, bv, bo, gq, gk))

    if not np.allclose(gk, 1.0):
        gkp = gk.reshape(-1, 2)
        assert np.allclose(gkp[:, 0], gkp[:, 1]), "unsupported non-pairwise gk"

    idx = np.flatnonzero(mask)
    nkeep = len(idx)
    skc = _skc_for(nkeep)
    base, rem = divmod(nkeep, N_CORES)
    counts = [base + (1 if c < rem else 0) for c in range(N_CORES)]
    offs = np.concatenate([[0], np.cumsum(counts)])

    def tile_w(WT):  # [1024,1024] (in,out of W.T) -> [p, i, o, m], scaled
        return np.ascontiguousarray(
            (WT * WS).reshape(8, 128, 8, 128).transpose(1, 0, 2, 3)).astype(f8)

    ii = np.arange(128)
    jj = ii // 2
    partner = ii ^ 1

    # fold gq (and pairwise gk) into the q rope tables; sin pairs with
    # partner's gq
    gq_t = (gq * gk).reshape(8, 128)
    gq_sin = (gq.reshape(8, 128)[:, partner] * gk.reshape(8, 128))
    cq = cos_q[:, jj].T                # [128, SQ]
    sq = sin_q[:, jj].T
    ctq = np.ascontiguousarray(
        (cq[None, :, :] * gq_t[:, :, None]).transpose(1, 0, 2)).astype(bf)
    stq = np.ascontiguousarray(
        (sq[None, :, :] * gq_sin[:, :, None]).transpose(1, 0, 2)).astype(bf)

    PT = np.zeros((128, 128), f)
    even = ii[ii % 2 == 0]
    PT[even + 1, even] = -1.0
    PT[even, even + 1] = 1.0

    selm = np.zeros((128, 2, 64), f)
    for h in range(8):
        selm[:, :, h * 8 + h] = 1.0

    # one-hot row lives on partition 0 only; this input IS partition 0's row
    qones = np.ones((1, 8, SQ), f).astype(f8)

    bo_f = bo + Wo @ bv

    shared = {
        "xt": np.ascontiguousarray(
            x.T.reshape(8, 128, SQ).transpose(1, 0, 2)).astype(f8),
        "wq": tile_w(Wq.T), "wk": tile_w(Wk.T), "wo": tile_w(Wo.T),
        "wv": np.ascontiguousarray(
            (Wv.T * WS).reshape(8, 128, DIM).transpose(1, 0, 2)).astype(f8),
        "ctq": ctq, "stq": stq,
        "bq_t": np.ascontiguousarray(bq.reshape(8, 128).T),
        "bk_t": np.ascontiguousarray(bk.reshape(8, 128).T),
        "bo_t": np.ascontiguousarray(bo_f.reshape(8, 128).T),
        "pmat": PT.astype(bf),
        "ones_c": np.ones((128, 1), bf),
        "sel": selm.astype(f8),
        "qones": qones,
        "eps_c": np.full((1, 1), EPS, f),
        "shf_c": np.full((128, 1), SHIFT, f),
    }

    ckT = cos_k[:, jj].T.astype(f)     # [128, SK]
    skT = sin_k[:, jj].T.astype(f)

    in_maps = []
    for c in range(N_CORES):
        keys = idx[offs[c]:offs[c + 1]]
        n = len(keys)
        m = dict(shared)
        memc = np.zeros((DIM, skc), f8)
        memc[:, :n] = mem[keys].T.astype(f8)
        ctkc = np.zeros((HD, skc), bf)
        stkc = np.zeros((HD, skc), bf)
        ctkc[:, :n] = ckT[:, keys].astype(bf)
        stkc[:, :n] = skT[:, keys].astype(bf)
        mb = np.full((skc,), MBPAD, f)
        mb[:n] = 0.0
        m["memT"] = memc
        m["ctk"] = np.ascontiguousarray(ctkc)
        m["stk"] = np.ascontiguousarray(stkc)
        m["mbr"] = np.ascontiguousarray(
            np.broadcast_to(mb[None, None, :], (1, HEADS, skc))).astype(f8)
        in_maps.append(m)
    return in_maps


def _assemble(outTs):
    parts = [np.asarray(outTs[c])[:, 0:QS].T for c in range(N_CORES)]
    out = np.concatenate(parts, axis=0)
    return out[None].astype(np.float32)


def kernel(**inputs):
    from concourse.bass_utils import run_bass_kernel_spmd
    in_maps = _prep(**inputs)
    skc = in_maps[0]["memT"].shape[1]
    nc = _build(skc)
    res = run_bass_kernel_spmd(nc, in_maps, list(range(N_CORES)))
    return _assemble([res.results[c]["outT"] for c in range(N_CORES)])
